# revision 16
# baseline (speedup 1.0000x reference)
"""Trainium2 Bass kernel for ConditionedStableKoopmanTransition.

Reference computation (per timestep, batched over B):
    z_{t+1} = z_t @ K^T + (u_t * dt) @ L^T          K = V diag(sig(e)*.99) V^-1
    y_t     = z_{t+1} @ C^T + (u_t * dt) @ Dm^T
    outputs Z = stack(z_1..z_T)  [T, B, 256],  Y = stack(y_0..y_T-1) [T, B, 80]

Strategy:
  - Data-parallel over batch: 8 cores x 256 rows each. Time recurrence local.
  - Host precomputes K (tiny dense algebra) and all derived weight products;
    y_t is rewritten in terms of z_t so every matmul of a step consumes the
    *previous* state.
  - Two-step unroll: one "pair" computes z_{t+1} (output only) and z_{t+2}
    (next state) both directly from z_t, using K^2 / K L / C K^2 / C K L
    weights precomputed on host. This takes the PSUM->SBUF state copy off
    the per-step critical path (it gets ~2us of slack instead of ~100ns)
    at the cost of 3 extra rank-48 matmuls per pair.
  - On-chip state is kept transposed ([d, batch]) so the recurrent matmul
    out = W.T @ state keeps weights stationary and state streaming; fp32r
    matmuls (full PE rate at N>=256, ~1e-4 rounding) with fp32 PSUM accum.
  - Grouped DMA stores (8 steps -> ~1MB contiguous transfers), u loads
    prefetched 4 chunks deep on a separate DMA path.
"""

import numpy as np
from contextlib import ExitStack

import concourse.bacc as bacc
import concourse.tile as tile
from concourse import mybir
from concourse.bass_utils import run_bass_kernel_spmd

T = 128
B = 2048
D = 256
UDIM = 48
NOBS = 80
NCORES = 8
BS = B // NCORES          # 256 batch rows per core
GROUP = 8                 # steps per z/y staging buffer (one DMA per group)
UCHUNK = 16               # steps per u-load DMA

_cache = {}


def _build(t_steps=T):
    key = ("nc", t_steps)
    if key in _cache:
        return _cache[key]
    f32r = mybir.dt.float32r
    f32 = mybir.dt.float32

    nc = bacc.Bacc("TRN2", target_bir_lowering=False, debug=False,
                   num_devices=NCORES)
    u_d = nc.dram_tensor("u", [UDIM, t_steps * BS], f32r, kind="ExternalInput").ap()
    z0_d = nc.dram_tensor("z0", [D, BS], f32r, kind="ExternalInput").ap()
    wz_d = nc.dram_tensor("wz", [D, D], f32r, kind="ExternalInput").ap()
    wz2_d = nc.dram_tensor("wz2", [D, D], f32r, kind="ExternalInput").ap()
    wl_d = nc.dram_tensor("wl", [UDIM, D], f32r, kind="ExternalInput").ap()
    wkl_d = nc.dram_tensor("wkl", [UDIM, D], f32r, kind="ExternalInput").ap()
    wy_d = nc.dram_tensor("wy", [D, NOBS], f32r, kind="ExternalInput").ap()
    wcd_d = nc.dram_tensor("wcd", [UDIM, NOBS], f32r, kind="ExternalInput").ap()
    # [h, j, t*BS + b]: per-partition rows are contiguous in DRAM across steps
    zout_d = nc.dram_tensor("zout", [2, 128, t_steps * BS], f32r,
                            kind="ExternalOutput").ap()
    yout_d = nc.dram_tensor("yout", [NOBS, t_steps * BS], f32,
                            kind="ExternalOutput").ap()

    n_groups = t_steps // GROUP
    uchunk = min(UCHUNK, t_steps)
    assert GROUP % 2 == 0 and t_steps % GROUP == 0

    with tile.TileContext(nc) as tc, ExitStack() as ctx:
        const = ctx.enter_context(tc.tile_pool(name="const", bufs=1))
        upool = ctx.enter_context(tc.tile_pool(name="upool", bufs=2))
        zstp = ctx.enter_context(tc.tile_pool(name="zstp", bufs=2))
        ystp = ctx.enter_context(tc.tile_pool(name="ystp", bufs=2))
        initp = ctx.enter_context(tc.tile_pool(name="initp", bufs=1))
        psz = ctx.enter_context(tc.tile_pool(name="psz", bufs=4, space="PSUM"))
        psy = ctx.enter_context(tc.tile_pool(name="psy", bufs=2, space="PSUM"))

        # init order: things the first pair needs come first (DMAs are
        # near-serial at startup)
        wl = const.tile([UDIM, D], f32r)
        nc.sync.dma_start(wl[:], wl_d[:])
        wkl = const.tile([UDIM, D], f32r)
        nc.sync.dma_start(wkl[:], wkl_d[:])
        wcd = const.tile([UDIM, NOBS], f32r)
        nc.sync.dma_start(wcd[:], wcd_d[:])
        s0i = initp.tile([128, BS], f32r)
        nc.sync.dma_start(s0i[:], z0_d[0:128, :])
        s1i = initp.tile([128, BS], f32r)
        nc.sync.dma_start(s1i[:], z0_d[128:256, :])
        wz0 = const.tile([128, D], f32r)
        nc.sync.dma_start(wz0[:], wz_d[0:128, :])
        wz1 = const.tile([128, D], f32r)
        nc.sync.dma_start(wz1[:], wz_d[128:256, :])
        wz20 = const.tile([128, D], f32r)
        nc.sync.dma_start(wz20[:], wz2_d[0:128, :])
        wz21 = const.tile([128, D], f32r)
        nc.sync.dma_start(wz21[:], wz2_d[128:256, :])
        wy0 = const.tile([128, NOBS], f32r)
        nc.sync.dma_start(wy0[:], wy_d[0:128, :])
        wy1 = const.tile([128, NOBS], f32r)
        nc.sync.dma_start(wy1[:], wy_d[128:256, :])
        cur0, cur1 = s0i[:], s1i[:]

        u_sb = None
        pending = None
        ypend = None
        for m in range(n_groups):
            zst0 = zstp.tile([128, GROUP * BS], f32r, tag="zst0", bufs=3)
            zst1 = zstp.tile([128, GROUP * BS], f32r, tag="zst1", bufs=3)
            yst = ystp.tile([NOBS, GROUP * BS], f32, tag="yst", bufs=3)
            if (m * GROUP) % uchunk == 0:
                c0 = m * GROUP * BS
                u_sb = upool.tile([UDIM, uchunk * BS], f32r, tag="u", bufs=4)
                if m == 0:
                    # first chunk split per-pair so early pairs aren't gated
                    # on one big DMA
                    for p0 in range(0, uchunk, 2):
                        nc.scalar.dma_start(u_sb[:, p0 * BS:(p0 + 2) * BS],
                                            u_d[:, p0 * BS:(p0 + 2) * BS])
                else:
                    # separate HWDGE ring (ACT) so loads don't queue behind stores
                    nc.scalar.dma_start(u_sb[:], u_d[:, c0:c0 + uchunk * BS])
            for k in range(0, GROUP, 2):
                t = m * GROUP + k
                ti = (t % uchunk) * BS
                uab = u_sb[:, ti:ti + 2 * BS]          # [u_t | u_{t+1}]
                ua = u_sb[:, ti:ti + BS]
                # combined psum per j-half: cols 0:256 = z_{t+1} ("a"),
                # cols 256:512 = z_{t+2} ("b", the next state)
                pz0 = psz.tile([128, 2 * BS], f32, tag="pz", bufs=4)
                pz1 = psz.tile([128, 2 * BS], f32, tag="pz", bufs=4)
                py = psy.tile([NOBS, 2 * BS], f32, tag="py", bufs=2)
                lo = slice(0, BS)
                hi = slice(BS, 2 * BS)
                # --- u-term matmuls (independent of state copies) ---
                # wl/wcd span both timesteps in one N=512 matmul.
                nc.tensor.matmul(pz0[:], wl[:, 0:128], uab, start=True, stop=False, skip_group_check=True)
                nc.tensor.matmul(pz1[:], wl[:, 128:256], uab, start=True, stop=False, skip_group_check=True)
                nc.tensor.matmul(py[:], wcd[:], uab, start=True, stop=False, skip_group_check=True)
                nc.tensor.matmul(pz0[:, hi], wkl[:, 0:128], ua, start=False, stop=False, skip_group_check=True)
                nc.tensor.matmul(pz1[:, hi], wkl[:, 128:256], ua, start=False, stop=False, skip_group_check=True)
                # --- deferred y_{t+1} of the previous pair: reads the staged
                # z_{t+1} (SBUF), whose copy completed during that pair ---
                if pending is not None:
                    ppy, psa0, psa1, pyst = pending
                    nc.tensor.matmul(ppy[:, BS:2 * BS], wy0[:], psa0, start=False, stop=False, skip_group_check=True)
                    nc.tensor.matmul(ppy[:, BS:2 * BS], wy1[:], psa1, start=False, stop=True, skip_group_check=True)
                    nc.scalar.copy(pyst, ppy[:])
                    pending = None
                # --- cur0-dependent ---
                nc.tensor.matmul(pz0[:, hi], wz20[:, 0:128], cur0, start=False, stop=False, skip_group_check=True)
                nc.tensor.matmul(pz1[:, hi], wz20[:, 128:256], cur0, start=False, stop=False, skip_group_check=True)
                nc.tensor.matmul(pz0[:, lo], wz0[:, 0:128], cur0, start=False, stop=False, skip_group_check=True)
                nc.tensor.matmul(pz1[:, lo], wz0[:, 128:256], cur0, start=False, stop=False, skip_group_check=True)
                nc.tensor.matmul(py[:, lo], wy0[:], cur0, start=False, stop=False, skip_group_check=True)
                # --- cur1-dependent (stops; "b" halves first: next state) ---
                nc.tensor.matmul(pz0[:, hi], wz21[:, 0:128], cur1, start=False, stop=True, skip_group_check=True)
                nc.tensor.matmul(pz1[:, hi], wz21[:, 128:256], cur1, start=False, stop=True, skip_group_check=True)
                nc.tensor.matmul(pz0[:, lo], wz1[:, 0:128], cur1, start=False, stop=True, skip_group_check=True)
                nc.tensor.matmul(pz1[:, lo], wz1[:, 128:256], cur1, start=False, stop=True, skip_group_check=True)
                nc.tensor.matmul(py[:, lo], wy1[:], cur1, start=False, stop=True, skip_group_check=True)
                # --- copies: one wide op per z j-half ---
                nc.vector.tensor_copy(zst0[:, k * BS:(k + 2) * BS], pz0[:])
                nc.vector.tensor_copy(zst1[:, k * BS:(k + 2) * BS], pz1[:])
                sa0 = zst0[:, k * BS:(k + 1) * BS]
                sa1 = zst1[:, k * BS:(k + 1) * BS]
                pending = (py, sa0, sa1, yst[:, k * BS:(k + 2) * BS])
                cur0 = zst0[:, (k + 1) * BS:(k + 2) * BS]
                cur1 = zst1[:, (k + 1) * BS:(k + 2) * BS]
            c0 = m * GROUP * BS
            c1 = (m + 1) * GROUP * BS
            if m == n_groups - 1 and pending is not None:
                ppy, psa0, psa1, pyst = pending
                nc.tensor.matmul(ppy[:, BS:2 * BS], wy0[:], psa0, start=False, stop=False, skip_group_check=True)
                nc.tensor.matmul(ppy[:, BS:2 * BS], wy1[:], psa1, start=False, stop=True, skip_group_check=True)
                nc.scalar.copy(pyst, ppy[:])
                pending = None
            nc.sync.dma_start(zout_d[0, :, c0:c1], zst0[:])
            nc.sync.dma_start(zout_d[1, :, c0:c1], zst1[:])
            # y store deferred one group: its last pair's y_hi copy is only
            # emitted inside the next group's first pair
            if ypend is not None:
                py0, pc0, pc1 = ypend
                nc.sync.dma_start(yout_d[:, pc0:pc1], py0[:])
            ypend = (yst, c0, c1)
            if m == n_groups - 1:
                nc.sync.dma_start(yout_d[:, c0:c1], yst[:])
                ypend = None

    nc.compile()
    _cache[key] = nc
    return nc


def _host_prep(z_dyn, dt, U, eig_raw, V, L, C, Dm, t_steps=T):
    z_dyn = np.asarray(z_dyn, dtype=np.float32)
    U = np.asarray(U, dtype=np.float32)
    dtv = float(np.asarray(dt, dtype=np.float32).reshape(-1)[0])
    e = np.asarray(eig_raw, dtype=np.float64)
    V64 = np.asarray(V, dtype=np.float64)
    L64 = np.asarray(L, dtype=np.float64)
    C64 = np.asarray(C, dtype=np.float64)
    D64 = np.asarray(Dm, dtype=np.float64)

    eig = 0.99 / (1.0 + np.exp(-e))
    K = (V64 * eig[None, :]) @ np.linalg.inv(V64)
    K2 = K @ K
    CK = C64 @ K
    CK2 = CK @ K
    CLD = (C64 @ L64 + D64) * dtv
    CKL = (CK @ L64) * dtv
    KL = (K @ L64) * dtv

    def t32(a):
        return np.ascontiguousarray(np.transpose(a)).astype(np.float32)

    wz = t32(K)          # [i, j] = K.T
    wz2 = t32(K2)
    wl = t32(L64 * dtv)  # [u, j]
    wkl = t32(KL)
    wy = t32(CK)         # [i, o]
    wcd = t32(CLD)       # [u, o]

    in_maps = []
    for c in range(NCORES):
        sl = slice(c * BS, (c + 1) * BS)
        u_c = np.ascontiguousarray(
            U[:t_steps, sl, :].transpose(2, 0, 1).reshape(UDIM, t_steps * BS))
        z0_c = np.ascontiguousarray(z_dyn[sl, :].T)
        in_maps.append({"u": u_c, "z0": z0_c, "wz": wz, "wz2": wz2,
                        "wl": wl, "wkl": wkl, "wy": wy, "wcd": wcd})
    return in_maps


def _gather(results, t_steps=T):
    Z = np.empty((t_steps, B, D), np.float32)
    Y = np.empty((t_steps, B, NOBS), np.float32)
    for c in range(NCORES):
        sl = slice(c * BS, (c + 1) * BS)
        zr = results[c]["zout"].reshape(2, 128, t_steps, BS)
        yr = results[c]["yout"].reshape(NOBS, t_steps, BS)
        # Z[t, b, h*128+j] = zr[h, j, t, b]
        Z[:, sl, :] = zr.transpose(2, 3, 0, 1).reshape(t_steps, BS, D)
        Y[:, sl, :] = yr.transpose(1, 2, 0)
    return Z, Y


def _run_device(in_maps, t_steps=T, **spmd_kwargs):
    nc = _build(t_steps)
    return run_bass_kernel_spmd(nc, in_maps, list(range(NCORES)), **spmd_kwargs)


def kernel(z_dyn, z_static, dt, U, eig_raw, V, L, C, Dm, **_unused):
    in_maps = _host_prep(z_dyn, dt, U, eig_raw, V, L, C, Dm)
    res = _run_device(in_maps)
    return _gather(res.results)
